# revision 35
# baseline (speedup 1.0000x reference)
"""Trainium2 Bass kernel for CaptionAttentionNet (fp8-hybrid version).

Model (B=128, T=64, V=10000, E=512, D=512, F=2048):
  h/c inits from image vectors; x = emb[captions_ix]
  h1s = LSTM1(x);  attn1 = out_proj1(v_proj1(h1s))        (softmax over 1 key == 1)
  h2s = LSTM2([h1s, attn1]);  attn2 = out_proj2(v_proj2(h2s))
  logits = [h2s, attn1, attn2] @ W_logits.T + b_logits

The affine "attention" folds into the weights on the host (attn_i = h_is @
M_i.T + a_i), so the device computes, per core (16 batch rows, t-major rows
row = t*16 + b):
  xp1 = x @ W_ih1r.T + b1          LSTM1 recurrence -> h1s
  xp2 = h1s @ Weff.T + b2eff       LSTM2 recurrence -> h2s
  logits = h1s @ G1.T + h2s @ G2.T (+ b_eff on host)

Precision plan (validated by host-side simulation, relmax ~6e-3 vs 2e-2 gate):
  - h magnitudes decay ~2x per step from ~0.9 (image init) to ~0.005, so the
    first 8 timesteps dominate both logits magnitude and quantization error.
  - logits m-block 0 (t<8) runs in bf16; m-blocks 1..7 run fp8-e4m3 with
    perf_mode=DoubleRow (FD=512, ~1.5x PE throughput).
  - xp1/xp2 run fp8 DoubleRow everywhere (error contribution tiny).
  - The LSTM recurrence is LDWEIGHTS-bound (FD=16): DoubleRow loses there,
    but plain fp8 weights halve the FWL load time.  Steps t<8 use bf16
    weights; t>=8 use fp8 weights with the bf16 h as moving operand.
  - Scales (power-of-2): weights x2048, x/h x128; gate psums land x2048
    (bf16 rec weights are pre-scaled x2048), xp tiles stored x2048,
    activations descale by 2^-11; fp8 logits psums land x2^18, descaled in
    the copy-out.  TRN fp8e4 clips at +-240.
"""

import os

if os.environ.get("JAX_PLATFORMS") == "cpu":
    os.environ.pop("JAX_PLATFORMS")

import numpy as np
import ml_dtypes

BF16 = ml_dtypes.bfloat16
FP8 = ml_dtypes.float8_e4m3fn

B, T, V, E, D, F = 128, 64, 10000, 512, 512, 2048
NCORES = 8
BC = B // NCORES  # 16 batch rows per core
R = BC * T  # 1024 t-major rows per core
VP = 10240  # padded vocab
NV = VP // 512  # 20 vocab chunks
NVP = NV // 2  # 10 v-pairs
G4 = 4 * D  # 2048 gates
SB = 8  # steps per block
NBLK = T // SB  # 8 row blocks of 128
L2LAG = 1  # L2 runs one step-block behind L1
RECBF = 8  # recurrence steps below this use bf16 weights and bf16 h

S_W = 2048.0  # weight scale (all fp8 weight tensors)
S_X = 128.0  # x fp8 scale (h fp8 copies are unscaled: |h|<1, subnormal
#              error on tiny late-t h is negligible in the logits)
S_PS = S_W  # gate-psum scale (bf16 rec weights pre-scaled by S_W)
S_GI = 1.0 / S_PS  # gate activation input scale
S_XPE1 = S_PS / (S_W * S_X)  # xp1 epilogue: psum x(S_W*S_X) -> stored xS_PS
S_XPE2 = 1.0  # xp2 epilogue: psum already x(S_W*1) = xS_PS
S_LG = 1.0 / S_W  # fp8 logits copy-out scale (h x1, G xS_W)

_GATE_PERM = [2, 0, 1, 3]  # (i, f, g, o) -> (g, i, f, o)


def _reorder_gates(w):
    return w.reshape(4, D, *w.shape[1:])[_GATE_PERM].reshape(4 * D, *w.shape[1:])


def _tt(w):
    """[G, K] -> [128, K//128, G] transposed k-chunk tiles (lhsT layout)."""
    g, k = w.shape
    return np.ascontiguousarray(w.T.reshape(k // 128, 128, g).transpose(1, 0, 2))


def _bt(v):
    """[BC, 512] -> [128, 4, BC] transposed chunk tiles."""
    return np.ascontiguousarray(v.T.reshape(4, 128, v.shape[0]).transpose(1, 0, 2))


def _fp8(v, scale):
    return np.clip(v * scale, -240.0, 240.0).astype(FP8)


def _host_prep(inputs):
    f32 = np.float32
    inp = {k: np.asarray(v) for k, v in inputs.items()}

    emb = inp["emb"].astype(f32)
    ix = inp["captions_ix"].astype(np.int64)
    img = inp["image_vectors"].astype(f32)

    x = emb[ix]  # [B, T, E]

    Wo1, Wv1 = inp["Wo1"].astype(f32), inp["Wv1"].astype(f32)
    Wo2, Wv2 = inp["Wo2"].astype(f32), inp["Wv2"].astype(f32)
    M1 = Wo1 @ Wv1
    a1b = inp["bo1"].astype(f32) + Wo1 @ inp["bv1"].astype(f32)
    M2 = Wo2 @ Wv2
    a2b = inp["bo2"].astype(f32) + Wo2 @ inp["bv2"].astype(f32)

    W_ih2 = inp["W_ih2"].astype(f32)
    Wa, Wb = W_ih2[:, :D], W_ih2[:, D:]
    Weff2 = Wa + Wb @ M1
    b2e = inp["b2"].astype(f32) + Wb @ a1b

    W_logits = inp["W_logits"].astype(f32)
    Wla, Wlb, Wlc = W_logits[:, :D], W_logits[:, D : 2 * D], W_logits[:, 2 * D :]
    G1 = Wlb @ M1
    G2 = Wla + Wlc @ M2
    blog = inp["b_logits"].astype(f32) + Wlb @ a1b + Wlc @ a2b

    h10 = img @ inp["W_init_h1"].astype(f32).T + inp["b_init_h1"].astype(f32)
    c10 = img @ inp["W_init_c1"].astype(f32).T + inp["b_init_c1"].astype(f32)
    h20 = img @ inp["W_init_h2"].astype(f32).T + inp["b_init_h2"].astype(f32)
    c20 = img @ inp["W_init_c2"].astype(f32).T + inp["b_init_c2"].astype(f32)

    wih1r = _reorder_gates(inp["W_ih1"].astype(f32))
    whh1r = _reorder_gates(inp["W_hh1"].astype(f32))
    whh2r = _reorder_gates(inp["W_hh2"].astype(f32))
    weff2r = _reorder_gates(Weff2)
    b1r = _reorder_gates(inp["b1"].astype(f32)[:, None])[:, 0]
    b2r = _reorder_gates(b2e[:, None])[:, 0]

    # G tiles.  bf16 (unscaled) for the m0 sweep: [NV, 128, 8, 512] with
    # [v, p, kc, n] = G12[v*512+n, kc*128+p] over the [VP, 1024] concat
    # [G1 | G2].  fp8 (scaled) paired for DoubleRow: [NV, 128, 4, 2, 512]
    # with [v, p, q, i, n] = G12[v*512+n, (2q+i)*128+p] * S_W.
    G12 = np.zeros((VP, 2 * D), f32)
    G12[:V, :D] = G1
    G12[:V, D:] = G2
    g12bf = np.ascontiguousarray(
        G12.T.reshape(8, 128, NV, 512).transpose(2, 1, 0, 3)
    ).astype(BF16)
    g12t8 = np.ascontiguousarray(
        _fp8(G12, S_W).reshape(VP, 4, 2, 128).transpose(3, 1, 2, 0)
        .reshape(128, 4, 2, NV, 512).transpose(3, 0, 1, 2, 4)
    )

    shared = {
        "wih1t8": _fp8(_tt(wih1r), S_W),
        "weff2t8": _fp8(_tt(weff2r), S_W),
        "whh1t8": _fp8(_tt(whh1r), S_W),
        "whh2t8": _fp8(_tt(whh2r), S_W),
        "whh1tb": (_tt(whh1r) * S_PS).astype(BF16),
        "whh2tb": (_tt(whh2r) * S_PS).astype(BF16),
        "b1g": np.ascontiguousarray(b1r.reshape(16, 128).T * S_PS).astype(f32),
        "b2g": np.ascontiguousarray(b2r.reshape(16, 128).T * S_PS).astype(f32),
        "g12bf": g12bf,
        "g12t8": g12t8,
    }

    per_core = []
    for c in range(NCORES):
        sl = slice(c * BC, (c + 1) * BC)
        xs = x[sl]  # [BC, T, E]
        xr = np.ascontiguousarray(xs.transpose(1, 0, 2)).reshape(R, E)
        xt = np.ascontiguousarray(xr.T.reshape(4, 128, R).transpose(1, 0, 2))
        per_core.append(
            {
                "xt8": _fp8(xt, S_X),
                "h1p0": _bt(h10[sl]).astype(BF16),
                "h2p0": _bt(h20[sl]).astype(BF16),
                "c10": _bt(c10[sl]).astype(f32),
                "c20": _bt(c20[sl]).astype(f32),
                **shared,
            }
        )
    return per_core, blog


def build_program(nc):
    import concourse.tile as tile
    from concourse import mybir

    dt = mybir.dt
    AF = mybir.ActivationFunctionType
    DR = mybir.MatmulPerfMode.DoubleRow

    def din(name, shape, dtype):
        return nc.dram_tensor(name, shape, dtype, kind="ExternalInput").ap()

    xt8_d = din("xt8", [128, 4, R], dt.float8e4)
    wih1t8_d = din("wih1t8", [128, 4, G4], dt.float8e4)
    weff2t8_d = din("weff2t8", [128, 4, G4], dt.float8e4)
    whh1t8_d = din("whh1t8", [128, 4, G4], dt.float8e4)
    whh2t8_d = din("whh2t8", [128, 4, G4], dt.float8e4)
    whh1tb_d = din("whh1tb", [128, 4, G4], dt.bfloat16)
    whh2tb_d = din("whh2tb", [128, 4, G4], dt.bfloat16)
    b1g_d = din("b1g", [128, 16], dt.float32)
    b2g_d = din("b2g", [128, 16], dt.float32)
    h1p0_d = din("h1p0", [128, 4, BC], dt.bfloat16)
    h2p0_d = din("h2p0", [128, 4, BC], dt.bfloat16)
    c10_d = din("c10", [128, 4, BC], dt.float32)
    c20_d = din("c20", [128, 4, BC], dt.float32)
    g12bf_d = din("g12bf", [NV, 128, 8, 512], dt.bfloat16)
    g12t8_d = din("g12t8", [NV, 128, 4, 2, 512], dt.float8e4)
    out_d = nc.dram_tensor("out", [R, V], dt.bfloat16, kind="ExternalOutput").ap()

    with tile.TileContext(nc) as tc:
        with (
            tc.tile_pool(name="const", bufs=1) as const,
            tc.tile_pool(name="state", bufs=1) as state,
            tc.tile_pool(name="work", bufs=5) as work,
            tc.tile_pool(name="gbuf8", bufs=4) as gbuf8,
            tc.tile_pool(name="gbufb", bufs=2) as gbufb,
            tc.tile_pool(name="obuf", bufs=4) as obuf,
            tc.tile_pool(name="pg", bufs=4, space="PSUM") as pg,
            tc.tile_pool(name="pl", bufs=4, space="PSUM") as pl,
        ):
            def load(pool, d_ap, shape, dtype, tag):
                t = pool.tile(shape, dtype, tag=tag)
                nc.sync.dma_start(out=t[:], in_=d_ap)
                return t

            # order matters: everything xp1 colblk 0 / LSTM1 step 0 needs first
            b1g = load(const, b1g_d[:], [128, 16], dt.float32, "b1g")
            h1p0 = load(const, h1p0_d[:], [128, 4, BC], dt.bfloat16, "h1p0")
            xt8 = const.tile([128, 4, R], dt.float8e4, tag="xt8")
            nc.sync.dma_start(out=xt8[:, :, 0:512], in_=xt8_d[:, :, 0:512])
            wih1t8 = load(const, wih1t8_d[:], [128, 4, G4], dt.float8e4, "wih1t8")
            whh1tb = load(const, whh1tb_d[:], [128, 4, G4], dt.bfloat16, "whh1tb")
            c1 = load(state, c10_d[:], [128, 4, BC], dt.float32, "c1")
            nc.sync.dma_start(out=xt8[:, :, 512:], in_=xt8_d[:, :, 512:])
            whh2tb = load(const, whh2tb_d[:], [128, 4, G4], dt.bfloat16, "whh2tb")
            weff2t8 = load(const, weff2t8_d[:], [128, 4, G4], dt.float8e4, "weff2t8")
            whh1t8 = load(const, whh1t8_d[:], [128, 4, G4], dt.float8e4, "whh1t8")
            whh2t8 = load(const, whh2t8_d[:], [128, 4, G4], dt.float8e4, "whh2t8")
            b2g = load(const, b2g_d[:], [128, 16], dt.float32, "b2g")
            h2p0 = load(const, h2p0_d[:], [128, 4, BC], dt.bfloat16, "h2p0")
            c2 = load(state, c20_d[:], [128, 4, BC], dt.float32, "c2")

            xp1t = state.tile([128, 16, R], dt.bfloat16, tag="xp1t")
            xp2t = state.tile([128, 16, R], dt.bfloat16, tag="xp2t")
            # bf16 h copies only exist for t<8 (bf16 rec steps + m0 logits)
            h1sb = state.tile([128, 4, RECBF * BC], dt.bfloat16, tag="h1sb")
            h2sb = state.tile([128, 4, RECBF * BC], dt.bfloat16, tag="h2sb")
            h1s8 = state.tile([128, 4, R], dt.float8e4, tag="h1s8")
            h2s8 = state.tile([128, 4, R], dt.float8e4, tag="h2s8")

            cc = [0]
            useq = [0]

            # ---- wide-matmul thunk queue ----
            widef = []

            def pump(n):
                for _ in range(min(n, len(widef))):
                    widef.pop(0)[2]()

            def drain_due(s):
                # xp1 block b (cols b*128..) feeds L1 block b at slot b
                rest = []
                for u in widef:
                    if u[0] == "xp1" and u[1] <= s:
                        u[2]()
                    else:
                        rest.append(u)
                widef[:] = rest

            # ---- one xp unit: 2 DoubleRow mms (k-pairs) + epilogue ----
            def push_xp(label, blk, wt8, rhs8, bg, xpt, gb, c0, width, scale):
                st = {}
                uid = useq[0]
                useq[0] += 1
                gsl = slice(gb * 128, (gb + 1) * 128)

                def mk(pc):
                    def th():
                        if pc == 0:
                            st["ps"] = pl.tile(
                                [128, 512], dt.float32, tag="pl",
                                name=f"plx{uid}",
                            )
                        nc.tensor.matmul(
                            st["ps"][:, :width],
                            wt8[:, 2 * pc : 2 * pc + 2, gsl],
                            rhs8[:, 2 * pc : 2 * pc + 2, c0 : c0 + width],
                            start=(pc == 0),
                            stop=(pc == 1),
                            perf_mode=DR,
                        )
                        if pc == 1:
                            cc[0] ^= 1
                            if cc[0]:
                                nc.scalar.activation(
                                    xpt[:, gb, c0 : c0 + width],
                                    st["ps"][:, :width],
                                    AF.Identity,
                                    bias=bg[:, gb : gb + 1],
                                    scale=scale,
                                )
                            else:
                                nc.vector.tensor_scalar(
                                    xpt[:, gb, c0 : c0 + width],
                                    st["ps"][:, :width],
                                    scale,
                                    bg[:, gb : gb + 1],
                                    mybir.AluOpType.mult,
                                    mybir.AluOpType.add,
                                )

                    return th

                for pc in range(2):
                    widef.append((label, blk, mk(pc)))

            # ---- LSTM recurrence, split into phases so an L1/L2 step pair
            # can interleave per-engine (strict-FIFO queues head-of-line
            # block otherwise: one layer's stalled op delays the other's
            # ready ops).  gates blocks: 0:4 = g, 4:8 = i, 8:12 = f,
            # 12:16 = o
            # gate psums for an L1/L2 step pair share one [128, 32, BC]
            # PSUM bank tile (PSUM allocation is bank-granular); `off` is
            # 0 (L1) or 16 (L2)
            def rec_mms(pst, off, t_, whhtb, hprevs, h0t):
                hp = (
                    h0t[:, :, :]
                    if t_ == 0
                    else hprevs[:, :, (t_ - 1) * BC : t_ * BC]
                )
                for gb in range(16):
                    gsl = slice(gb * 128, (gb + 1) * 128)
                    for dc in range(4):
                        nc.tensor.matmul(
                            pst[:, off + gb, :],
                            whhtb[:, dc, gsl],
                            hp[:, dc, :],
                            start=(dc == 0),
                            stop=(dc == 3),
                        )

            def rec_adds(pst, off, t_, xpt):
                xps = xpt[:, :, t_ * BC : (t_ + 1) * BC]
                nc.vector.tensor_add(
                    pst[:, off : off + 4, :], pst[:, off : off + 4, :],
                    xps[:, :4, :],
                )
                nc.vector.tensor_add(
                    pst[:, off + 4 : off + 16, :],
                    pst[:, off + 4 : off + 16, :], xps[:, 4:, :],
                )

            def rec_acts(pst, off):
                tg = work.tile([128, 4, BC], dt.float32, tag="tg")
                nc.scalar.activation(
                    tg[:], pst[:, off : off + 4, :], AF.Tanh, scale=S_GI
                )
                ss = work.tile([128, 12, BC], dt.float32, tag="ss")
                nc.scalar.activation(
                    ss[:], pst[:, off + 4 : off + 16, :], AF.Sigmoid, scale=S_GI
                )
                return tg, ss

            def rec_cupd(tg, ss, c):
                t1 = work.tile([128, 4, BC], dt.float32, tag="t1")
                nc.vector.tensor_mul(t1[:], ss[:, 4:8, :], c[:])
                t2 = work.tile([128, 4, BC], dt.float32, tag="t2")
                nc.vector.tensor_mul(t2[:], ss[:, :4, :], tg[:])
                nc.vector.tensor_add(c[:], t1[:], t2[:])

            def rec_tanhc(c):
                tc_ = work.tile([128, 4, BC], dt.float32, tag="tc")
                nc.scalar.activation(tc_[:], c[:], AF.Tanh)
                return tc_

            def rec_hout(t_, ss, tc_, hsb, hs8):
                # h = sigmoid(o) * tanh(c).  t<8: full-precision bf16 h
                # first (bf16 rec steps + m0 logits read it), fp8 cast
                # second; t>=8: only the fp8 copy exists, write it directly.
                hcols = slice(t_ * BC, (t_ + 1) * BC)
                if t_ < RECBF:
                    nc.vector.tensor_mul(hsb[:, :, hcols], ss[:, 8:12, :], tc_[:])
                    if t_ % 2 == 0:
                        nc.scalar.copy(hs8[:, :, hcols], hsb[:, :, hcols])
                    else:
                        nc.vector.tensor_copy(hs8[:, :, hcols], hsb[:, :, hcols])
                else:
                    nc.vector.tensor_mul(hs8[:, :, hcols], ss[:, 8:12, :], tc_[:])

            def lstm_step(t_, wtb, wt8, xpt, hsb, hs8, h0t, c):
                wt, hx = (wtb, hsb) if t_ < RECBF else (wt8, hs8)
                pst = pg.tile([128, 16, BC], dt.float32, tag="pg")
                rec_mms(pst, 0, t_, wt, hx, h0t)
                rec_adds(pst, 0, t_, xpt)
                tg, ss = rec_acts(pst, 0)
                rec_cupd(tg, ss, c)
                tc_ = rec_tanhc(c)
                rec_hout(t_, ss, tc_, hsb, hs8)

            def lstm_pair(t1_, t2_):
                # L1 step t1_ and L2 step t2_, engine queues interleaved
                w1, hx1 = (whh1tb, h1sb) if t1_ < RECBF else (whh1t8, h1s8)
                w2, hx2 = (whh2tb, h2sb) if t2_ < RECBF else (whh2t8, h2s8)
                ps1 = pg.tile([128, 16, BC], dt.float32, tag="pg")
                ps2 = pg.tile([128, 16, BC], dt.float32, tag="pg")
                rec_mms(ps1, 0, t1_, w1, hx1, h1p0)
                rec_mms(ps2, 0, t2_, w2, hx2, h2p0)
                rec_adds(ps1, 0, t1_, xp1t)
                a1 = rec_acts(ps1, 0)
                rec_adds(ps2, 0, t2_, xp2t)
                a2 = rec_acts(ps2, 0)
                rec_cupd(a1[0], a1[1], c1)
                tc1 = rec_tanhc(c1)
                rec_cupd(a2[0], a2[1], c2)
                tc2 = rec_tanhc(c2)
                rec_hout(t1_, a1[1], tc1, h1sb, h1s8)
                rec_hout(t2_, a2[1], tc2, h2sb, h2s8)

            # ---- fp8 logits v-pair (vp, m>=1): 8 DR mms as thunks ----
            def push_pair8(vp, m, gt0, gt1):
                st = {}
                msl = slice(m * 128, (m + 1) * 128)
                uid = useq[0]
                useq[0] += 1

                def mk(unit, p, v, gt, col):
                    def th():
                        if p == 0:
                            st[unit] = pl.tile(
                                [128, 512], dt.float32, tag="pl",
                                name=f"plp{uid}_{unit}",
                            )
                            if unit == 0:
                                st["ot"] = obuf.tile(
                                    [128, 1024], dt.bfloat16, tag="otp",
                                    name=f"otp{uid}",
                                )
                        ps = st[unit]
                        hs8 = h1s8 if p < 2 else h2s8
                        q = p % 2
                        nc.tensor.matmul(
                            ps[:],
                            hs8[:, 2 * q : 2 * q + 2, msl],
                            gt[:, p, :, :],
                            start=(p == 0),
                            stop=(p == 3),
                            perf_mode=DR,
                        )
                        if p == 3:
                            width = min(512, V - v * 512)
                            cc[0] ^= 1
                            if cc[0]:
                                nc.scalar.activation(
                                    st["ot"][:, col : col + width],
                                    ps[:, :width],
                                    AF.Copy,
                                    scale=S_LG,
                                )
                            else:
                                nc.vector.tensor_scalar_mul(
                                    st["ot"][:, col : col + width],
                                    ps[:, :width],
                                    S_LG,
                                )
                            if unit == 1:
                                w = 512 + width
                                nc.sync.dma_start(
                                    out=out_d[msl, vp * 1024 : vp * 1024 + w],
                                    in_=st["ot"][:, :w],
                                )

                    return th

                for p in range(4):
                    widef.append(("lg", None, mk(0, p, 2 * vp, gt0, 0)))
                for p in range(4):
                    widef.append(("lg", None, mk(1, p, 2 * vp + 1, gt1, 512)))

            # ---- bf16 logits v-pair for m-block 0: 16 bf16 mms as thunks ----
            def push_pairb(vp, gt0, gt1):
                st = {}
                uid = useq[0]
                useq[0] += 1

                def mk(unit, kc, v, gt, col):
                    def th():
                        if kc == 0:
                            st[unit] = pl.tile(
                                [128, 512], dt.float32, tag="pl",
                                name=f"plb{uid}_{unit}",
                            )
                            if unit == 0:
                                st["ot"] = obuf.tile(
                                    [128, 1024], dt.bfloat16, tag="otp",
                                    name=f"otb{uid}",
                                )
                        ps = st[unit]
                        hs = h1sb if kc < 4 else h2sb
                        nc.tensor.matmul(
                            ps[:],
                            hs[:, kc % 4, 0:128],
                            gt[:, kc, :],
                            start=(kc == 0),
                            stop=(kc == 7),
                        )
                        if kc == 7:
                            width = min(512, V - v * 512)
                            cc[0] ^= 1
                            if cc[0]:
                                nc.scalar.copy(
                                    st["ot"][:, col : col + width], ps[:, :width]
                                )
                            else:
                                nc.vector.tensor_copy(
                                    st["ot"][:, col : col + width], ps[:, :width]
                                )
                            if unit == 1:
                                w = 512 + width
                                nc.sync.dma_start(
                                    out=out_d[0:128, vp * 1024 : vp * 1024 + w],
                                    in_=st["ot"][:, :w],
                                )

                    return th

                for kc in range(8):
                    widef.append(("lg", None, mk(0, kc, 2 * vp, gt0, 0)))
                for kc in range(8):
                    widef.append(("lg", None, mk(1, kc, 2 * vp + 1, gt1, 512)))

            # ---- gt tile loads ----
            gts8 = {}
            gtsb = {}
            gseq = [0]

            def load_pair8(vp):
                k = gseq[0]
                gseq[0] += 1
                g0 = gbuf8.tile([128, 4, 2, 512], dt.float8e4, tag="gt8", name=f"g8{k}a")
                nc.sync.dma_start(out=g0[:], in_=g12t8_d[2 * vp])
                g1 = gbuf8.tile([128, 4, 2, 512], dt.float8e4, tag="gt8", name=f"g8{k}b")
                nc.sync.dma_start(out=g1[:], in_=g12t8_d[2 * vp + 1])
                gts8[vp] = (g0, g1)

            def load_pairb(vp):
                k = gseq[0]
                gseq[0] += 1
                g0 = gbufb.tile([128, 8, 512], dt.bfloat16, tag="gtb", name=f"gb{k}a")
                nc.sync.dma_start(out=g0[:], in_=g12bf_d[2 * vp])
                g1 = gbufb.tile([128, 8, 512], dt.bfloat16, tag="gtb", name=f"gb{k}b")
                nc.sync.dma_start(out=g1[:], in_=g12bf_d[2 * vp + 1])
                gtsb[vp] = (g0, g1)

            # ---- phase 1: xp1 block 0 (cols 0:128), direct emission ----
            for gb in range(16):
                push_xp("xp1", 0, wih1t8, xt8, b1g, xp1t, gb, 0, 128, S_XPE1)
            pump(32)  # L1 step 0's xp add needs all 16 gate blocks

            # ---- fill plan: (minslot, unit) in consumption order ----
            # slot s runs L1 block s and L2 block s-1.  L2 block m is done
            # at slot m+1, so fp8 logits m-block m is ready from slot m+2
            # (m>=1); m0-bf16 pairs ready from slot 2.  Fills are paced into
            # the PE-idle window of each recurrence iteration (the chain
            # crawl), NOT front-loaded: over-pumping delays the recurrence,
            # under-pumping leaves the PE idle and re-throttles the HAM
            # clock gate.
            NSLOT = NBLK + L2LAG
            fill_plan = []
            # xp1 blocks 1..7 (16 units each): block b is consumed by L1
            # block b at slot b; emit from slot b-1 so early slots always
            # have fill inventory
            for b in range(1, NBLK):
                for gb in range(16):
                    fill_plan.append((b - 1, ("xp1u", b, gb)))
            # m0 bf16 pairs: two per slot over slots 2..6
            for vp in range(NVP):
                fill_plan.append((2 + vp // 2, ("loadb", vp)))
                fill_plan.append((2 + vp // 2, ("lgbp", vp)))
            # fp8 pair waves (runs per vp amortize the gt tile loads):
            # A: m 1-2 from slot 4; B: m 3-4 from slot 6; C: m 5-6 from
            # slot 8; m 7 runs in the tail.
            done_pairs = set()
            for lo, hi, ms in ((1, 2, 4), (3, 4, 6), (5, 6, 8)):
                for vp in range(NVP):
                    fill_plan.append((ms, ("load8", vp)))
                    for m in range(lo, hi + 1):
                        fill_plan.append((ms, ("lgp", vp, m)))
                        done_pairs.add((vp, m))
            fill_plan.sort(key=lambda e: e[0])  # stable: keeps in-slot order

            def emit_fill(u):
                if u[0] == "xp1u":
                    push_xp("xp1", u[1], wih1t8, xt8, b1g, xp1t, u[2],
                            u[1] * 128, 128, S_XPE1)
                elif u[0] == "load8":
                    load_pair8(u[1])
                elif u[0] == "loadb":
                    load_pairb(u[1])
                elif u[0] == "lgbp":
                    push_pairb(u[1], *gtsb[u[1]])
                else:
                    _, vp, m = u
                    push_pair8(vp, m, *gts8[vp])

            def l1_step(t_):
                lstm_step(t_, whh1tb, whh1t8, xp1t, h1sb, h1s8, h1p0, c1)

            def l2_step(t_):
                lstm_step(t_, whh2tb, whh2t8, xp2t, h2sb, h2s8, h2p0, c2)

            PUMP_ITER = 12
            for s in range(NSLOT):
                drain_due(s)
                for i in range(SB):
                    if s < NBLK and s >= L2LAG:
                        lstm_pair(s * SB + i, (s - L2LAG) * SB + i)
                    elif s < NBLK:
                        l1_step(s * SB + i)
                    else:
                        l2_step((s - L2LAG) * SB + i)
                    # refill the wide queue from the plan, then pump the
                    # iteration's PE-idle budget (wides land after this
                    # iteration's rec mms in PE program order)
                    while (
                        fill_plan
                        and fill_plan[0][0] <= s
                        and len(widef) < 24
                    ):
                        emit_fill(fill_plan.pop(0)[1])
                    pump(PUMP_ITER)
                # xp2 for L1 block s (just produced); L2 block s consumes it
                # next slot, so emit its units now
                if s < NBLK:
                    for gb in range(16):
                        push_xp("xp2", s, weff2t8, h1s8, b2g, xp2t, gb,
                                s * 128, 128, S_XPE2)
                    rest = []
                    for u in widef:
                        if u[0] == "xp2" and u[1] <= s:
                            u[2]()
                        else:
                            rest.append(u)
                    widef[:] = rest
            for _, u in fill_plan:
                emit_fill(u)
            pump(len(widef))

            # ---- tail: remaining fp8 logits pairs ----
            tail_vps = [
                vp
                for vp in range(NVP)
                if any((vp, m) not in done_pairs for m in range(1, NBLK))
            ]
            if tail_vps:
                load_pair8(tail_vps[0])
            for i, vp in enumerate(tail_vps):
                todo = [m for m in range(1, NBLK) if (vp, m) not in done_pairs]
                for j, m in enumerate(todo):
                    push_pair8(vp, m, *gts8[vp])
                    if j == 0 and i + 1 < len(tail_vps):
                        load_pair8(tail_vps[i + 1])
                    pump(len(widef))
    return out_d


_CACHED = {}


def _get_compiled():
    if "nc" not in _CACHED:
        from concourse import bacc

        nc = bacc.Bacc(
            "TRN2", target_bir_lowering=False, debug=False, num_devices=NCORES
        )
        build_program(nc)
        nc.compile()
        _CACHED["nc"] = nc
    return _CACHED["nc"]


def kernel(**inputs):
    from concourse.bass_utils import run_bass_kernel_spmd

    per_core, blog = _host_prep(inputs)
    nc = _get_compiled()
    res = run_bass_kernel_spmd(nc, per_core, list(range(NCORES)))
    outs = []
    for c in range(NCORES):
        o = res.results[c]["out"].astype(np.float32).reshape(T, BC, V)
        outs.append(o.transpose(1, 0, 2))
    out = np.concatenate(outs, axis=0).reshape(B, T, V)
    out += blog[None, None, :].astype(np.float32)
    return out


# revision 43
# speedup vs baseline: 1.1925x; 1.1925x over previous
"""Trainium2 Bass kernel for CaptionAttentionNet (fp8-hybrid version).

Model (B=128, T=64, V=10000, E=512, D=512, F=2048):
  h/c inits from image vectors; x = emb[captions_ix]
  h1s = LSTM1(x);  attn1 = out_proj1(v_proj1(h1s))        (softmax over 1 key == 1)
  h2s = LSTM2([h1s, attn1]);  attn2 = out_proj2(v_proj2(h2s))
  logits = [h2s, attn1, attn2] @ W_logits.T + b_logits

The affine "attention" folds into the weights on the host (attn_i = h_is @
M_i.T + a_i), so the device computes, per core (16 batch rows, t-major rows
row = t*16 + b):
  xp1 = x @ W_ih1r.T + b1          LSTM1 recurrence -> h1s
  xp2 = h1s @ Weff.T + b2eff       LSTM2 recurrence -> h2s
  logits = h1s @ G1.T + h2s @ G2.T (+ b_eff on host)

Precision plan (validated by host-side simulation, relmax ~6e-3 vs 2e-2 gate):
  - h magnitudes decay ~2x per step from ~0.9 (image init) to ~0.005, so the
    first 8 timesteps dominate both logits magnitude and quantization error.
  - logits m-block 0 (t<8) runs in bf16; m-blocks 1..7 run fp8-e4m3 with
    perf_mode=DoubleRow (FD=512, ~1.5x PE throughput).
  - xp1/xp2 run fp8 DoubleRow everywhere (error contribution tiny).
  - The LSTM recurrence is LDWEIGHTS-bound (FD=16): DoubleRow loses there,
    but plain fp8 weights halve the FWL load time.  Steps t<8 use bf16
    weights; t>=8 use fp8 weights with the bf16 h as moving operand.
  - Scales (power-of-2): weights x2048, x/h x128; gate psums land x2048
    (bf16 rec weights are pre-scaled x2048), xp tiles stored x2048,
    activations descale by 2^-11; fp8 logits psums land x2^18, descaled in
    the copy-out.  TRN fp8e4 clips at +-240.
"""

import os

if os.environ.get("JAX_PLATFORMS") == "cpu":
    os.environ.pop("JAX_PLATFORMS")

import numpy as np
import ml_dtypes

BF16 = ml_dtypes.bfloat16
FP8 = ml_dtypes.float8_e4m3fn

B, T, V, E, D, F = 128, 64, 10000, 512, 512, 2048
NCORES = 8
BC = B // NCORES  # 16 batch rows per core
R = BC * T  # 1024 t-major rows per core
VP = 10240  # padded vocab
NV = VP // 512  # 20 vocab chunks
NVP = NV // 2  # 10 v-pairs
G4 = 4 * D  # 2048 gates
SB = 8  # steps per block
NBLK = T // SB  # 8 row blocks of 128
L2LAG = 1  # L2 runs one step-block behind L1
RECBF = 8  # recurrence steps below this use bf16 weights and bf16 h

S_W = 2048.0  # weight scale (all fp8 weight tensors)
S_X = 128.0  # x fp8 scale (h fp8 copies are unscaled: |h|<1, subnormal
#              error on tiny late-t h is negligible in the logits)
S_PS = S_W  # gate-psum scale (bf16 rec weights pre-scaled by S_W)
S_GI = 1.0 / S_PS  # gate activation input scale
S_XPE1 = S_PS / (S_W * S_X)  # xp1 epilogue: psum x(S_W*S_X) -> stored xS_PS
S_XPE2 = 1.0  # xp2 epilogue: psum already x(S_W*1) = xS_PS
S_LG = 1.0 / S_W  # fp8 logits copy-out scale (h x1, G xS_W)

_GATE_PERM = [2, 0, 1, 3]  # (i, f, g, o) -> (g, i, f, o)


def _reorder_gates(w):
    return w.reshape(4, D, *w.shape[1:])[_GATE_PERM].reshape(4 * D, *w.shape[1:])


def _tt(w):
    """[G, K] -> [128, K//128, G] transposed k-chunk tiles (lhsT layout)."""
    g, k = w.shape
    return np.ascontiguousarray(w.T.reshape(k // 128, 128, g).transpose(1, 0, 2))


def _bt(v):
    """[BC, 512] -> [128, 4, BC] transposed chunk tiles."""
    return np.ascontiguousarray(v.T.reshape(4, 128, v.shape[0]).transpose(1, 0, 2))


def _fp8(v, scale):
    return np.clip(v * scale, -240.0, 240.0).astype(FP8)


def _host_prep(inputs):
    f32 = np.float32
    inp = {k: np.asarray(v) for k, v in inputs.items()}

    emb = inp["emb"].astype(f32)
    ix = inp["captions_ix"].astype(np.int64)
    img = inp["image_vectors"].astype(f32)

    x = emb[ix]  # [B, T, E]

    Wo1, Wv1 = inp["Wo1"].astype(f32), inp["Wv1"].astype(f32)
    Wo2, Wv2 = inp["Wo2"].astype(f32), inp["Wv2"].astype(f32)
    M1 = Wo1 @ Wv1
    a1b = inp["bo1"].astype(f32) + Wo1 @ inp["bv1"].astype(f32)
    M2 = Wo2 @ Wv2
    a2b = inp["bo2"].astype(f32) + Wo2 @ inp["bv2"].astype(f32)

    W_ih2 = inp["W_ih2"].astype(f32)
    Wa, Wb = W_ih2[:, :D], W_ih2[:, D:]
    Weff2 = Wa + Wb @ M1
    b2e = inp["b2"].astype(f32) + Wb @ a1b

    W_logits = inp["W_logits"].astype(f32)
    Wla, Wlb, Wlc = W_logits[:, :D], W_logits[:, D : 2 * D], W_logits[:, 2 * D :]
    G1 = Wlb @ M1
    G2 = Wla + Wlc @ M2
    blog = inp["b_logits"].astype(f32) + Wlb @ a1b + Wlc @ a2b

    h10 = img @ inp["W_init_h1"].astype(f32).T + inp["b_init_h1"].astype(f32)
    c10 = img @ inp["W_init_c1"].astype(f32).T + inp["b_init_c1"].astype(f32)
    h20 = img @ inp["W_init_h2"].astype(f32).T + inp["b_init_h2"].astype(f32)
    c20 = img @ inp["W_init_c2"].astype(f32).T + inp["b_init_c2"].astype(f32)

    wih1r = _reorder_gates(inp["W_ih1"].astype(f32))
    whh1r = _reorder_gates(inp["W_hh1"].astype(f32))
    whh2r = _reorder_gates(inp["W_hh2"].astype(f32))
    weff2r = _reorder_gates(Weff2)
    b1r = _reorder_gates(inp["b1"].astype(f32)[:, None])[:, 0]
    b2r = _reorder_gates(b2e[:, None])[:, 0]

    # G tiles.  bf16 (unscaled) for the m0 sweep: [NV, 128, 8, 512] with
    # [v, p, kc, n] = G12[v*512+n, kc*128+p] over the [VP, 1024] concat
    # [G1 | G2].  fp8 (scaled) paired for DoubleRow: [NV, 128, 4, 2, 512]
    # with [v, p, q, i, n] = G12[v*512+n, (2q+i)*128+p] * S_W.
    G12 = np.zeros((VP, 2 * D), f32)
    G12[:V, :D] = G1
    G12[:V, D:] = G2
    g12bf = np.ascontiguousarray(
        G12.T.reshape(8, 128, NV, 512).transpose(2, 1, 0, 3)
    ).astype(BF16)
    g12t8 = np.ascontiguousarray(
        _fp8(G12, S_W).reshape(VP, 4, 2, 128).transpose(3, 1, 2, 0)
        .reshape(128, 4, 2, NV, 512).transpose(3, 0, 1, 2, 4)
    )

    shared = {
        "ident": np.eye(128, dtype=np.float32).astype(BF16),
        "wih1t8": _fp8(_tt(wih1r), S_W),
        "weff2t8": _fp8(_tt(weff2r), S_W),
        "whh1t8": _fp8(_tt(whh1r), S_W),
        "whh2t8": _fp8(_tt(whh2r), S_W),
        "whh1tb": (_tt(whh1r) * S_PS).astype(BF16),
        "whh2tb": (_tt(whh2r) * S_PS).astype(BF16),
        "b1g": np.ascontiguousarray(b1r.reshape(16, 128).T * S_PS).astype(f32),
        "b2g": np.ascontiguousarray(b2r.reshape(16, 128).T * S_PS).astype(f32),
        "g12bf": g12bf,
        "g12t8": g12t8,
    }

    per_core = []
    for c in range(NCORES):
        sl = slice(c * BC, (c + 1) * BC)
        xs = x[sl]  # [BC, T, E]
        xr = np.ascontiguousarray(xs.transpose(1, 0, 2)).reshape(R, E)
        xt = np.ascontiguousarray(xr.T.reshape(4, 128, R).transpose(1, 0, 2))
        per_core.append(
            {
                "xt8": _fp8(xt, S_X),
                "h1p0": _bt(h10[sl]).astype(BF16),
                "h2p0": _bt(h20[sl]).astype(BF16),
                "c10": _bt(c10[sl]).astype(f32),
                "c20": _bt(c20[sl]).astype(f32),
                **shared,
            }
        )
    return per_core, blog


def build_program(nc):
    import concourse.tile as tile
    from concourse import mybir

    dt = mybir.dt
    AF = mybir.ActivationFunctionType
    DR = mybir.MatmulPerfMode.DoubleRow

    def din(name, shape, dtype):
        return nc.dram_tensor(name, shape, dtype, kind="ExternalInput").ap()

    xt8_d = din("xt8", [128, 4, R], dt.float8e4)
    ident_d = din("ident", [128, 128], dt.bfloat16)
    wih1t8_d = din("wih1t8", [128, 4, G4], dt.float8e4)
    weff2t8_d = din("weff2t8", [128, 4, G4], dt.float8e4)
    whh1t8_d = din("whh1t8", [128, 4, G4], dt.float8e4)
    whh2t8_d = din("whh2t8", [128, 4, G4], dt.float8e4)
    whh1tb_d = din("whh1tb", [128, 4, G4], dt.bfloat16)
    whh2tb_d = din("whh2tb", [128, 4, G4], dt.bfloat16)
    b1g_d = din("b1g", [128, 16], dt.float32)
    b2g_d = din("b2g", [128, 16], dt.float32)
    h1p0_d = din("h1p0", [128, 4, BC], dt.bfloat16)
    h2p0_d = din("h2p0", [128, 4, BC], dt.bfloat16)
    c10_d = din("c10", [128, 4, BC], dt.float32)
    c20_d = din("c20", [128, 4, BC], dt.float32)
    g12bf_d = din("g12bf", [NV, 128, 8, 512], dt.bfloat16)
    g12t8_d = din("g12t8", [NV, 128, 4, 2, 512], dt.float8e4)
    out_d = nc.dram_tensor("out", [R, V], dt.bfloat16, kind="ExternalOutput").ap()

    with tile.TileContext(nc) as tc:
        with (
            tc.tile_pool(name="const", bufs=1) as const,
            tc.tile_pool(name="state", bufs=1) as state,
            tc.tile_pool(name="work", bufs=5) as work,
            tc.tile_pool(name="gbuf8", bufs=4) as gbuf8,
            tc.tile_pool(name="gbufb", bufs=2) as gbufb,
            tc.tile_pool(name="obuf", bufs=4) as obuf,
            tc.tile_pool(name="pg", bufs=4, space="PSUM") as pg,
            tc.tile_pool(name="pl", bufs=4, space="PSUM") as pl,
        ):
            def load(pool, d_ap, shape, dtype, tag):
                t = pool.tile(shape, dtype, tag=tag)
                nc.sync.dma_start(out=t[:], in_=d_ap)
                return t

            # order matters: everything xp1 colblk 0 / LSTM1 step 0 needs first
            ident = load(const, ident_d[:], [128, 128], dt.bfloat16, "ident")
            b1g = load(const, b1g_d[:], [128, 16], dt.float32, "b1g")
            h1p0 = load(const, h1p0_d[:], [128, 4, BC], dt.bfloat16, "h1p0")
            xt8 = const.tile([128, 4, R], dt.float8e4, tag="xt8")
            nc.sync.dma_start(out=xt8[:, :, 0:512], in_=xt8_d[:, :, 0:512])
            wih1t8 = load(const, wih1t8_d[:], [128, 4, G4], dt.float8e4, "wih1t8")
            whh1tb = load(const, whh1tb_d[:], [128, 4, G4], dt.bfloat16, "whh1tb")
            c1 = load(state, c10_d[:], [128, 4, BC], dt.float32, "c1")
            nc.sync.dma_start(out=xt8[:, :, 512:], in_=xt8_d[:, :, 512:])
            whh2tb = load(const, whh2tb_d[:], [128, 4, G4], dt.bfloat16, "whh2tb")
            weff2t8 = load(const, weff2t8_d[:], [128, 4, G4], dt.float8e4, "weff2t8")
            whh1t8 = load(const, whh1t8_d[:], [128, 4, G4], dt.float8e4, "whh1t8")
            whh2t8 = load(const, whh2t8_d[:], [128, 4, G4], dt.float8e4, "whh2t8")
            b2g = load(const, b2g_d[:], [128, 16], dt.float32, "b2g")
            h2p0 = load(const, h2p0_d[:], [128, 4, BC], dt.bfloat16, "h2p0")
            c2 = load(state, c20_d[:], [128, 4, BC], dt.float32, "c2")

            xp1t = state.tile([128, 16, R], dt.bfloat16, tag="xp1t")
            xp2t = state.tile([128, 16, R], dt.bfloat16, tag="xp2t")
            # bf16 h copies only exist for t<8 (bf16 rec steps + m0 logits)
            h1sb = state.tile([128, 4, RECBF * BC], dt.bfloat16, tag="h1sb")
            h2sb = state.tile([128, 4, RECBF * BC], dt.bfloat16, tag="h2sb")
            h1s8 = state.tile([128, 4, R], dt.float8e4, tag="h1s8")
            h2s8 = state.tile([128, 4, R], dt.float8e4, tag="h2s8")

            cc = [0]
            useq = [0]

            # ---- wide-matmul thunk queue ----
            widef = []

            def pump(n):
                for _ in range(min(n, len(widef))):
                    widef.pop(0)[2]()

            def drain_due(s):
                # xp1 colblk c (cols c*512..) feeds L1 blocks 4c..4c+3
                rest = []
                for u in widef:
                    if u[0] == "xp1" and 4 * u[1] <= s:
                        u[2]()
                    else:
                        rest.append(u)
                widef[:] = rest

            # ---- one xp unit: 2 DoubleRow mms (k-pairs) + epilogue ----
            def push_xp(label, blk, wt8, rhs8, bg, xpt, gb, c0, width, scale):
                st = {}
                uid = useq[0]
                useq[0] += 1
                gsl = slice(gb * 128, (gb + 1) * 128)

                def mk(pc):
                    def th():
                        if pc == 0:
                            st["ps"] = pl.tile(
                                [128, 512], dt.float32, tag="pl",
                                name=f"plx{uid}",
                            )
                        nc.tensor.matmul(
                            st["ps"][:, :width],
                            wt8[:, 2 * pc : 2 * pc + 2, gsl],
                            rhs8[:, 2 * pc : 2 * pc + 2, c0 : c0 + width],
                            start=(pc == 0),
                            stop=(pc == 1),
                            perf_mode=DR,
                        )
                        if pc == 1:
                            cc[0] ^= 1
                            if cc[0]:
                                nc.scalar.activation(
                                    xpt[:, gb, c0 : c0 + width],
                                    st["ps"][:, :width],
                                    AF.Identity,
                                    bias=bg[:, gb : gb + 1],
                                    scale=scale,
                                )
                            else:
                                nc.vector.tensor_scalar(
                                    xpt[:, gb, c0 : c0 + width],
                                    st["ps"][:, :width],
                                    scale,
                                    bg[:, gb : gb + 1],
                                    mybir.AluOpType.mult,
                                    mybir.AluOpType.add,
                                )

                    return th

                for pc in range(2):
                    widef.append((label, blk, mk(pc)))

            # ---- LSTM recurrence, split into phases so an L1/L2 step pair
            # can interleave per-engine (strict-FIFO queues head-of-line
            # block otherwise: one layer's stalled op delays the other's
            # ready ops).  gates blocks: 0:4 = g, 4:8 = i, 8:12 = f,
            # 12:16 = o
            # One recurrence step's matmuls.  The xp slice is summed into
            # the gate psum by a single identity matmul (start of the
            # accumulation group) so no vector add is needed and the
            # activations read PSUM directly — one less cross-engine hop
            # on the recurrence critical path.
            def rec_mms(pst, t_, whhtb, hprevs, h0t, xpt):
                hp = (
                    h0t[:, :, :]
                    if t_ == 0
                    else hprevs[:, :, (t_ - 1) * BC : t_ * BC]
                )
                nc.tensor.matmul(
                    pst[:, :, :],
                    ident[:, :],
                    xpt[:, :, t_ * BC : (t_ + 1) * BC],
                    start=True,
                    stop=False,
                )
                for gb in range(16):
                    gsl = slice(gb * 128, (gb + 1) * 128)
                    for dc in range(4):
                        nc.tensor.matmul(
                            pst[:, gb, :],
                            whhtb[:, dc, gsl],
                            hp[:, dc, :],
                            start=False,
                            stop=(dc == 3),
                        )

            def rec_acts(pst):
                tg = work.tile([128, 4, BC], dt.float32, tag="tg")
                nc.scalar.activation(
                    tg[:], pst[:, :4, :], AF.Tanh, scale=S_GI
                )
                ss = work.tile([128, 12, BC], dt.float32, tag="ss")
                nc.scalar.activation(
                    ss[:], pst[:, 4:, :], AF.Sigmoid, scale=S_GI
                )
                return tg, ss

            def rec_cupd(tg, ss, c):
                t1 = work.tile([128, 4, BC], dt.float32, tag="t1")
                nc.vector.tensor_mul(t1[:], ss[:, 4:8, :], c[:])
                t2 = work.tile([128, 4, BC], dt.float32, tag="t2")
                nc.vector.tensor_mul(t2[:], ss[:, :4, :], tg[:])
                nc.vector.tensor_add(c[:], t1[:], t2[:])

            def rec_tanhc(c):
                tc_ = work.tile([128, 4, BC], dt.float32, tag="tc")
                nc.scalar.activation(tc_[:], c[:], AF.Tanh)
                return tc_

            def rec_hout(t_, ss, tc_, hsb, hs8):
                # h = sigmoid(o) * tanh(c).  t<8: full-precision bf16 h
                # first (bf16 rec steps + m0 logits read it), fp8 cast
                # second; t>=8: only the fp8 copy exists, write it directly.
                hcols = slice(t_ * BC, (t_ + 1) * BC)
                if t_ < RECBF:
                    nc.vector.tensor_mul(hsb[:, :, hcols], ss[:, 8:12, :], tc_[:])
                    if t_ % 2 == 0:
                        nc.scalar.copy(hs8[:, :, hcols], hsb[:, :, hcols])
                    else:
                        nc.vector.tensor_copy(hs8[:, :, hcols], hsb[:, :, hcols])
                else:
                    nc.vector.tensor_mul(hs8[:, :, hcols], ss[:, 8:12, :], tc_[:])

            def lstm_step(t_, wtb, wt8, xpt, hsb, hs8, h0t, c):
                wt, hx = (wtb, hsb) if t_ < RECBF else (wt8, hs8)
                pst = pg.tile([128, 16, BC], dt.float32, tag="pg")
                rec_mms(pst, t_, wt, hx, h0t, xpt)
                tg, ss = rec_acts(pst)
                rec_cupd(tg, ss, c)
                tc_ = rec_tanhc(c)
                rec_hout(t_, ss, tc_, hsb, hs8)

            def lstm_pair(t1_, t2_):
                # L1 step t1_ and L2 step t2_, engine queues interleaved
                w1, hx1 = (whh1tb, h1sb) if t1_ < RECBF else (whh1t8, h1s8)
                w2, hx2 = (whh2tb, h2sb) if t2_ < RECBF else (whh2t8, h2s8)
                ps1 = pg.tile([128, 16, BC], dt.float32, tag="pg")
                ps2 = pg.tile([128, 16, BC], dt.float32, tag="pg")
                rec_mms(ps1, t1_, w1, hx1, h1p0, xp1t)
                rec_mms(ps2, t2_, w2, hx2, h2p0, xp2t)
                a1 = rec_acts(ps1)
                a2 = rec_acts(ps2)
                rec_cupd(a1[0], a1[1], c1)
                tc1 = rec_tanhc(c1)
                rec_cupd(a2[0], a2[1], c2)
                tc2 = rec_tanhc(c2)
                rec_hout(t1_, a1[1], tc1, h1sb, h1s8)
                rec_hout(t2_, a2[1], tc2, h2sb, h2s8)

            # ---- fp8 logits v-pair (vp, m>=1): 8 DR mms as thunks ----
            def push_pair8(vp, m, gt0, gt1):
                st = {}
                msl = slice(m * 128, (m + 1) * 128)
                uid = useq[0]
                useq[0] += 1

                def mk(unit, p, v, gt, col):
                    def th():
                        if p == 0:
                            st[unit] = pl.tile(
                                [128, 512], dt.float32, tag="pl",
                                name=f"plp{uid}_{unit}",
                            )
                            if unit == 0:
                                st["ot"] = obuf.tile(
                                    [128, 1024], dt.bfloat16, tag="otp",
                                    name=f"otp{uid}",
                                )
                        ps = st[unit]
                        hs8 = h1s8 if p < 2 else h2s8
                        q = p % 2
                        nc.tensor.matmul(
                            ps[:],
                            hs8[:, 2 * q : 2 * q + 2, msl],
                            gt[:, p, :, :],
                            start=(p == 0),
                            stop=(p == 3),
                            perf_mode=DR,
                        )
                        if p == 3:
                            width = min(512, V - v * 512)
                            cc[0] ^= 1
                            if cc[0]:
                                nc.scalar.activation(
                                    st["ot"][:, col : col + width],
                                    ps[:, :width],
                                    AF.Copy,
                                    scale=S_LG,
                                )
                            else:
                                nc.vector.tensor_scalar_mul(
                                    st["ot"][:, col : col + width],
                                    ps[:, :width],
                                    S_LG,
                                )
                            if unit == 1:
                                w = 512 + width
                                nc.sync.dma_start(
                                    out=out_d[msl, vp * 1024 : vp * 1024 + w],
                                    in_=st["ot"][:, :w],
                                )

                    return th

                for p in range(4):
                    widef.append(("lg", None, mk(0, p, 2 * vp, gt0, 0)))
                for p in range(4):
                    widef.append(("lg", None, mk(1, p, 2 * vp + 1, gt1, 512)))

            # ---- bf16 logits v-pair for m-block 0: 16 bf16 mms as thunks ----
            def push_pairb(vp, gt0, gt1):
                st = {}
                uid = useq[0]
                useq[0] += 1

                def mk(unit, kc, v, gt, col):
                    def th():
                        if kc == 0:
                            st[unit] = pl.tile(
                                [128, 512], dt.float32, tag="pl",
                                name=f"plb{uid}_{unit}",
                            )
                            if unit == 0:
                                st["ot"] = obuf.tile(
                                    [128, 1024], dt.bfloat16, tag="otp",
                                    name=f"otb{uid}",
                                )
                        ps = st[unit]
                        hs = h1sb if kc < 4 else h2sb
                        nc.tensor.matmul(
                            ps[:],
                            hs[:, kc % 4, 0:128],
                            gt[:, kc, :],
                            start=(kc == 0),
                            stop=(kc == 7),
                        )
                        if kc == 7:
                            width = min(512, V - v * 512)
                            cc[0] ^= 1
                            if cc[0]:
                                nc.scalar.copy(
                                    st["ot"][:, col : col + width], ps[:, :width]
                                )
                            else:
                                nc.vector.tensor_copy(
                                    st["ot"][:, col : col + width], ps[:, :width]
                                )
                            if unit == 1:
                                w = 512 + width
                                nc.sync.dma_start(
                                    out=out_d[0:128, vp * 1024 : vp * 1024 + w],
                                    in_=st["ot"][:, :w],
                                )

                    return th

                for kc in range(8):
                    widef.append(("lg", None, mk(0, kc, 2 * vp, gt0, 0)))
                for kc in range(8):
                    widef.append(("lg", None, mk(1, kc, 2 * vp + 1, gt1, 512)))

            # ---- gt tile loads ----
            gts8 = {}
            gtsb = {}
            gseq = [0]

            def load_pair8(vp):
                k = gseq[0]
                gseq[0] += 1
                g0 = gbuf8.tile([128, 4, 2, 512], dt.float8e4, tag="gt8", name=f"g8{k}a")
                nc.sync.dma_start(out=g0[:], in_=g12t8_d[2 * vp])
                g1 = gbuf8.tile([128, 4, 2, 512], dt.float8e4, tag="gt8", name=f"g8{k}b")
                nc.sync.dma_start(out=g1[:], in_=g12t8_d[2 * vp + 1])
                gts8[vp] = (g0, g1)

            def load_pairb(vp):
                k = gseq[0]
                gseq[0] += 1
                g0 = gbufb.tile([128, 8, 512], dt.bfloat16, tag="gtb", name=f"gb{k}a")
                nc.sync.dma_start(out=g0[:], in_=g12bf_d[2 * vp])
                g1 = gbufb.tile([128, 8, 512], dt.bfloat16, tag="gtb", name=f"gb{k}b")
                nc.sync.dma_start(out=g1[:], in_=g12bf_d[2 * vp + 1])
                gtsb[vp] = (g0, g1)

            # ---- phase 1: xp1 colblk 0 (cols 0:512), direct emission ----
            for gb in range(16):
                push_xp("xp1", 0, wih1t8, xt8, b1g, xp1t, gb, 0, 512, S_XPE1)
            pump(32)  # L1 step 0's identity-mm needs all 16 gate blocks

            # ---- fill plan: (minslot, unit) in consumption order ----
            # slot s runs L1 block s and L2 block s-1.  L2 block m is done
            # at slot m+1, so fp8 logits m-block m is ready from slot m+2
            # (m>=1); m0-bf16 pairs ready from slot 2.  Fills are paced into
            # the PE-idle window of each recurrence iteration (the chain
            # crawl), NOT front-loaded: over-pumping delays the recurrence,
            # under-pumping leaves the PE idle and re-throttles the HAM
            # clock gate.
            NSLOT = NBLK + L2LAG
            fill_plan = []
            # xp1 colblk 1 (cols 512:1024): due at slot 4
            for gb in range(16):
                fill_plan.append((0, ("xp1u", gb)))
            # m0 bf16 pairs: two per slot over slots 2..6
            for vp in range(NVP):
                fill_plan.append((2 + vp // 2, ("loadb", vp)))
                fill_plan.append((2 + vp // 2, ("lgbp", vp)))
            # fp8 pair waves (runs per vp amortize the gt tile loads):
            # A: m 1-2 from slot 4; B: m 3-4 from slot 6; m 5-7 in the tail
            done_pairs = set()
            for lo, hi, ms in ((1, 2, 4), (3, 4, 6)):
                for vp in range(NVP):
                    fill_plan.append((ms, ("load8", vp)))
                    for m in range(lo, hi + 1):
                        fill_plan.append((ms, ("lgp", vp, m)))
                        done_pairs.add((vp, m))
            fill_plan.sort(key=lambda e: e[0])  # stable: keeps in-slot order

            def emit_fill(u):
                if u[0] == "xp1u":
                    push_xp("xp1", 1, wih1t8, xt8, b1g, xp1t, u[1],
                            512, 512, S_XPE1)
                elif u[0] == "load8":
                    load_pair8(u[1])
                elif u[0] == "loadb":
                    load_pairb(u[1])
                elif u[0] == "lgbp":
                    push_pairb(u[1], *gtsb[u[1]])
                else:
                    _, vp, m = u
                    push_pair8(vp, m, *gts8[vp])

            def l1_step(t_):
                lstm_step(t_, whh1tb, whh1t8, xp1t, h1sb, h1s8, h1p0, c1)

            def l2_step(t_):
                lstm_step(t_, whh2tb, whh2t8, xp2t, h2sb, h2s8, h2p0, c2)

            PUMP_ITER = 12
            for s in range(NSLOT):
                drain_due(s)
                for i in range(SB):
                    if s < NBLK and s >= L2LAG:
                        lstm_pair(s * SB + i, (s - L2LAG) * SB + i)
                    elif s < NBLK:
                        l1_step(s * SB + i)
                    else:
                        l2_step((s - L2LAG) * SB + i)
                    # refill the wide queue from the plan, then pump the
                    # iteration's PE-idle budget (wides land after this
                    # iteration's rec mms in PE program order)
                    while (
                        fill_plan
                        and fill_plan[0][0] <= s
                        and len(widef) < 24
                    ):
                        emit_fill(fill_plan.pop(0)[1])
                    pump(PUMP_ITER)
                # xp2 for L1 block s (just produced); L2 block s consumes it
                # next slot, so emit its units now
                if s < NBLK:
                    for gb in range(16):
                        push_xp("xp2", s, weff2t8, h1s8, b2g, xp2t, gb,
                                s * 128, 128, S_XPE2)
                    rest = []
                    for u in widef:
                        if u[0] == "xp2" and u[1] <= s:
                            u[2]()
                        else:
                            rest.append(u)
                    widef[:] = rest
            for _, u in fill_plan:
                emit_fill(u)
            pump(len(widef))

            # ---- tail: remaining fp8 logits pairs ----
            tail_vps = [
                vp
                for vp in range(NVP)
                if any((vp, m) not in done_pairs for m in range(1, NBLK))
            ]
            if tail_vps:
                load_pair8(tail_vps[0])
            for i, vp in enumerate(tail_vps):
                todo = [m for m in range(1, NBLK) if (vp, m) not in done_pairs]
                for j, m in enumerate(todo):
                    push_pair8(vp, m, *gts8[vp])
                    if j == 0 and i + 1 < len(tail_vps):
                        load_pair8(tail_vps[i + 1])
                    pump(len(widef))
    return out_d


_CACHED = {}


def _get_compiled():
    if "nc" not in _CACHED:
        from concourse import bacc

        nc = bacc.Bacc(
            "TRN2", target_bir_lowering=False, debug=False, num_devices=NCORES
        )
        build_program(nc)
        nc.compile()
        _CACHED["nc"] = nc
    return _CACHED["nc"]


def kernel(**inputs):
    from concourse.bass_utils import run_bass_kernel_spmd

    per_core, blog = _host_prep(inputs)
    nc = _get_compiled()
    res = run_bass_kernel_spmd(nc, per_core, list(range(NCORES)))
    outs = []
    for c in range(NCORES):
        o = res.results[c]["out"].astype(np.float32).reshape(T, BC, V)
        outs.append(o.transpose(1, 0, 2))
    out = np.concatenate(outs, axis=0).reshape(B, T, V)
    out += blog[None, None, :].astype(np.float32)
    return out
